# revision 28
# baseline (speedup 1.0000x reference)
"""KAN layer (uniform cubic B-spline, grid=8, k=3) Trainium2 kernel, v3.

Math
----
Reference computes, per batch row n and output o:
    out[n,o] = sum_i w_silu[i,o]*silu(x[n,i]) + sum_i w_sp[i,o] * sum_b B_b(x[n,i]) * C[b,i,o]

With the uniform knot grid t_j = -1.75 + 0.25*j, put s = 4x+7 in [3,11). The
spline space restricted to [-1,1] is spanned by 11 truncated-power functions
{1, x, x^2, x^3} u {(s-k)_+^3/6 : k=7..10} u {(k-s)_+^3/6 : k=4..6} (the
two-sided split keeps every tile O(1)-bounded so reduced-precision matmul
stays well-conditioned). silu(x) itself is least-squares fitted in the SAME
basis (max abs fit err 2.9e-6), so w_silu folds into the 11 weight groups and
needs no group of its own. The device computes 11 activation tiles and
contracts them against (n_in, n_out) weight groups on the PE, fp32 PSUM.

Every cube tile is produced sign-positive in three ops with no clamp op:
    t2 = (s-k)^2/6          [ACT Square with affine]
    q  = (s-k) * t2         [DVE scalar_tensor_tensor]  ( = (s-k)^3/6 )
    cube = relu(+-q)        [ACT Relu, scale=-1 for left knots]

Mixed precision (host-simulated end-to-end rel err 1.31e-2 vs 2e-2 budget):
  - x shipped bf16; P0 (ones) and P1 (=x) groups bf16.
  - P2, P3, R7 groups fp32r (1 PE cycle/row, same speed as bf16).
  - L6, R8, L5, R9 cube chains + weights fully bf16 (2x elementwise
    throughput, half the weight DMA).
  - Tiny cubes L4+R10 packed into ONE fp8e4m3 DoubleRow matmul group
    (two contraction planes per instruction), tiles scaled 2^10, weights
    2^12, accumulated in a second PSUM bank set, merged at the end as
    out = psum_main + 2^-22 * psum_fp8.

Schedule: all weight DMAs issued up-front to persistent SBUF tiles in
first-use order (no buffer-recycle stalls). `ones` needs no DMA and a dozen
bf16 scratch matmuls warm the PE p-state during the DMA lead-in. Engine
queues are strict FIFO, so ACT/DVE emission order is interleaved to land
each tile just before its matmul group. Output row-chunks are merged and
DMA'd as soon as their last matmul retires.

Sharding: data-parallel over batch N across 8 cores (512 rows each);
weights replicated. No collectives.
"""

import numpy as np
import ml_dtypes

N, N_IN, N_OUT = 4096, 512, 512
NB = 11
NCORES = 8
ROWS = N // NCORES          # batch rows per core
G = N_IN // 128             # 4 partition groups over n_in
M = ROWS // 128             # 4 PSUM row-chunks
W5 = (1.0, -4.0, 6.0, -4.0, 1.0)

# silu(x) ~ lstsq fit in the truncated-power basis
# [1, x, x^2, x^3, L4, L5, L6, R7, R8, R9, R10]  (max abs err 2.9e-6)
SILU_COEF = np.array([
    -2.85017504e-06, 5.00000000e-01, 2.51316134e-01, 1.04215478e-02,
    -6.77741053e-04, -1.30248882e-03, -1.77424080e-03, -1.95404022e-03,
    -1.77424080e-03, -1.30248882e-03, -6.77741053e-04,
])

# cube groups in matmul-group order: (kind, knot, chain dtype)
CUBES_R = [("L", 6, "bf"), ("R", 7, "f32"), ("R", 8, "bf"), ("L", 5, "bf"), ("R", 9, "bf")]
CUBES_8 = [("L", 4), ("R", 10)]   # fp8 DoubleRow pair
A_SH = 10   # fp8 tile scale 2^A_SH
B_SH = 12   # fp8 weight scale 2^B_SH
N_WARMUP = 7

_CACHE = {}


def _fp32r(a):
    """Round float32 array to fp32r (11-bit mantissa, RNE) — matches the
    walrus fp32_to_fp32r semantics so device-side rounding is a no-op."""
    a = np.ascontiguousarray(a, dtype=np.float32)
    bits = a.view(np.uint32)
    rnd = ((bits >> np.uint32(12)) & np.uint32(1)) + np.uint32(0x7FF)
    return ((bits + rnd) & np.uint32(0xFFFFF000)).view(np.float32)


def _poly_alpha():
    """alpha[j, t]: coefficient of x^t in the polynomial part of B_j."""
    alpha = np.zeros((NB, 4), dtype=np.float64)
    for j in range(NB):
        for p in range(5):
            k = j + p
            if k <= 6:  # (s-k)^3/6 with s-k = 4x + (7-k)
                a = 7.0 - k
                alpha[j, 3] += W5[p] * 64.0 / 6.0
                alpha[j, 2] += W5[p] * 48.0 * a / 6.0
                alpha[j, 1] += W5[p] * 12.0 * a * a / 6.0
                alpha[j, 0] += W5[p] * a * a * a / 6.0
    return alpha


def _prep_weights(C, w_silu, w_sp):
    """Fold C*w_sp and the silu fit into the 11 weight groups.
    Returns (BROW bf16 [1,n_out] = colsum of the constant group, WP1 bf16,
    WF f32r [P2,P3,R7], WB bf16 [L6,R8,L5,R9], W8 fp8 [n_in, 2, n_out])."""
    Ceff = C.astype(np.float64) * w_sp.astype(np.float64)[None]
    ws = w_silu.astype(np.float64)
    alpha = _poly_alpha()
    beta = np.einsum("jt,jio->tio", alpha, Ceff)  # (4, n_in, n_out)
    Wp = [beta[t] + SILU_COEF[t] * ws for t in range(4)]

    cube_order = [("L", 4), ("L", 5), ("L", 6), ("R", 7), ("R", 8), ("R", 9), ("R", 10)]
    Wc = {}
    for gi, (kind, k) in enumerate(cube_order):
        # device tiles are (+-(s-k))_+^3/6, always positive: no sign flips
        wk = np.zeros((N_IN, N_OUT), dtype=np.float64)
        for p in range(5):
            j = k - p
            if 0 <= j < NB:
                wk += W5[p] * Ceff[j]
        Wc[(kind, k)] = wk + SILU_COEF[4 + gi] * ws

    bf = ml_dtypes.bfloat16
    BROW = Wp[0].sum(axis=0, keepdims=True).astype(np.float32).astype(bf)
    WP1 = Wp[1].astype(np.float32).astype(bf)
    WF = _fp32r(np.stack([Wp[2], Wp[3], Wc[("R", 7)]]).astype(np.float32))
    WB = np.stack([Wc[("L", 6)], Wc[("R", 8)], Wc[("L", 5)], Wc[("R", 9)]])
    WB = WB.astype(np.float32).astype(bf)
    W8 = np.empty((N_IN, 2, N_OUT), dtype=np.float32)
    W8[:, 0] = Wc[CUBES_8[0]] * 2.0 ** B_SH
    W8[:, 1] = Wc[CUBES_8[1]] * 2.0 ** B_SH
    W8 = W8.astype(ml_dtypes.float8_e4m3)
    return BROW, WP1, WF, WB, W8


def _build():
    import concourse.bacc as bacc
    import concourse.mybir as mybir
    from concourse import tile

    f32 = mybir.dt.float32
    f32r = mybir.dt.float32r
    bf16 = mybir.dt.bfloat16
    fp8 = mybir.dt.float8e4
    AF = mybir.ActivationFunctionType
    ALU = mybir.AluOpType
    DR = mybir.MatmulPerfMode.DoubleRow

    c2 = 6.0 ** (-0.5)          # sqrt scaling: t2 = (c2*(s-k))^2
    s8 = 2.0 ** (A_SH / 2)      # extra sqrt scale for the fp8 pair

    nc = bacc.Bacc("TRN2", target_bir_lowering=False, debug=False)
    XT = nc.dram_tensor("xT", [N_IN, ROWS], bf16, kind="ExternalInput").ap()
    TB = nc.dram_tensor("Brow", [1, N_OUT], bf16, kind="ExternalInput").ap()
    TP1 = nc.dram_tensor("Wp1", [N_IN, N_OUT], bf16, kind="ExternalInput").ap()
    TWF = nc.dram_tensor("Wf", [3, N_IN, N_OUT], f32r, kind="ExternalInput").ap()
    TWB = nc.dram_tensor("Wb", [4, N_IN, N_OUT], bf16, kind="ExternalInput").ap()
    TW8 = nc.dram_tensor("W8", [N_IN, 2, N_OUT], fp8, kind="ExternalInput").ap()
    OUT = nc.dram_tensor("out", [ROWS, N_OUT], f32, kind="ExternalOutput").ap()

    with tile.TileContext(nc) as tc:
        with (
            tc.tile_pool(name="const", bufs=1) as constp,
            tc.tile_pool(name="t2f", bufs=1) as t2fp,
            tc.tile_pool(name="t2b", bufs=4) as t2bp,
            tc.tile_pool(name="qf", bufs=1) as qfp,
            tc.tile_pool(name="qb", bufs=3) as qbp,
            tc.tile_pool(name="cubef", bufs=1) as cubefp,
            tc.tile_pool(name="cubeb", bufs=3) as cubebp,
            tc.tile_pool(name="outp", bufs=6) as outp,
            tc.tile_pool(name="psp", bufs=1, space="PSUM") as psp,
        ):
            # ---- persistent tiles ----
            xt = constp.tile([128, G, ROWS], bf16)
            brow = constp.tile([1, N_OUT], bf16)
            dqP1 = constp.tile([128, G, N_OUT], bf16)
            dqF = constp.tile([128, 3, G, N_OUT], f32r)
            dqB = constp.tile([128, 4, G, N_OUT], bf16)
            dq8 = constp.tile([128, G, 2, N_OUT], fp8)
            ones = constp.tile([128, 128], bf16)
            scratch = constp.tile([128, N_OUT], f32)
            scr16 = constp.tile([128, N_OUT], bf16)
            U = constp.tile([128, G, ROWS], f32)
            p2 = constp.tile([128, G, ROWS], f32r)
            p3 = constp.tile([128, G, ROWS], f32r)
            cube8 = constp.tile([128, 2, G, ROWS], fp8)

            # ---- all DMAs up-front, one instruction per tensor (each extra
            # dma_start costs ~600ns of Sync issue time), in first-use order;
            # within a multi-plane tensor the DMA delivers planes in order ----
            nc.sync.dma_start(brow[:], TB[:])
            nc.sync.dma_start(xt[:], XT.rearrange("(g p) n -> p g n", p=128))
            nc.sync.dma_start(dqP1[:], TP1.rearrange("(g p) o -> p g o", p=128))
            nc.sync.dma_start(dqF[:], TWF.rearrange("j (g p) o -> p j g o", p=128))
            nc.sync.dma_start(dqB[:], TWB.rearrange("j (g p) o -> p j g o", p=128))
            nc.sync.dma_start(dq8[:], TW8.rearrange("(g p) two o -> p g two o", p=128))

            # ---- constants (no DMA deps) + PE warm-up ----
            nc.gpsimd.memset(scratch[:], 0.0)
            # bf16/f32r memset fails the walrus ISA check; derive from scratch
            nc.vector.tensor_scalar(ones[:], scratch[:, 0:128], 0.0, 1.0,
                                    op0=ALU.mult, op1=ALU.add)
            nc.vector.tensor_scalar(scr16[:], scratch[:], 0.0, 0.0,
                                    op0=ALU.mult, op1=ALU.add)
            # ACT Square bias columns: 5 f32r/bf16 cubes then the 2 fp8 cubes
            nbias = len(CUBES_R) + len(CUBES_8)
            bias_t2 = constp.tile([128, nbias], f32)
            for ci, (kind, k, _dt) in enumerate(CUBES_R):
                nc.gpsimd.memset(bias_t2[:, ci:ci + 1], (7.0 - k) * c2)
            for idx, (kind, k) in enumerate(CUBES_8):
                nc.gpsimd.memset(bias_t2[:, 5 + idx:5 + idx + 1], (7.0 - k) * c2 * s8)

            psm = [psp.tile([128, N_OUT], f32, name=f"ps{m}", tag=f"ps{m}") for m in range(M)]
            ps8 = [psp.tile([128, N_OUT], f32, name=f"q{m}", tag=f"q{m}") for m in range(M)]

            for _ in range(N_WARMUP):
                nc.tensor.matmul(ps8[M - 1][:], ones[:], scr16[:],
                                 start=True, stop=True)

            # ---- activation tiles (FIFO engines: order is the schedule) ----
            cube_specs = {}  # ci -> (kind, k, t2 tile, q pool, cube pool/dtype)

            def emit_sq(bcol, kind, k, dt):
                pool = t2fp if dt == "f32" else t2bp
                t2 = pool.tile([128, G, ROWS], f32 if dt == "f32" else bf16,
                               name="t2", tag=f"t2{dt}")
                sc = 4.0 * c2 * (s8 if dt == "fp8" else 1.0)
                nc.scalar.activation(t2[:], xt[:], AF.Square,
                                     bias=bias_t2[:, bcol:bcol + 1], scale=sc)
                return t2

            def emit_stt(k, t2, dt):
                pool = qfp if dt == "f32" else qbp
                q = pool.tile([128, G, ROWS], f32 if dt == "f32" else bf16,
                              name="q", tag=f"q{dt}")
                nc.vector.scalar_tensor_tensor(q[:], U[:], -float(k), t2[:],
                                               op0=ALU.add, op1=ALU.mult)
                return q

            def emit_relu(kind, q, dst):
                nc.scalar.activation(dst, q[:], AF.Relu,
                                     scale=(-1.0 if kind == "L" else 1.0))

            # ACT: p2 chunks; DVE: U
            for g in range(G):
                nc.scalar.activation(p2[:, g, :], xt[:, g, :], AF.Square)
            nc.vector.tensor_scalar(U[:], xt[:], 4.0, 7.0, op0=ALU.mult, op1=ALU.add)

            # ACT: first squares while DVE does p3
            sq = {}
            sq[0] = emit_sq(0, *CUBES_R[0][:2], CUBES_R[0][2])
            sq[1] = emit_sq(1, *CUBES_R[1][:2], CUBES_R[1][2])
            for g in range(G):
                nc.vector.tensor_tensor(p3[:, g, :], p2[:, g, :], xt[:, g, :], op=ALU.mult)
            sq[2] = emit_sq(2, *CUBES_R[2][:2], CUBES_R[2][2])
            sq[3] = emit_sq(3, *CUBES_R[3][:2], CUBES_R[3][2])

            # DVE stt + ACT relu pipeline, interleaved with remaining squares
            cube_tiles = [None] * len(CUBES_R)

            def make_cube(ci):
                kind, k, dt = CUBES_R[ci]
                q = emit_stt(k, sq[ci], dt)
                pool = cubefp if dt == "f32" else cubebp
                cube = pool.tile([128, G, ROWS], f32r if dt == "f32" else bf16,
                                 name="cube", tag=f"cube{dt}")
                emit_relu(kind, q[:], cube[:])
                cube_tiles[ci] = cube

            make_cube(0)                                  # L6
            make_cube(1)                                  # R7
            sq[4] = emit_sq(4, *CUBES_R[4][:2], CUBES_R[4][2])
            make_cube(2)                                  # R8
            sq[5] = emit_sq(5, *CUBES_8[0], "fp8")
            make_cube(3)                                  # L5
            sq[6] = emit_sq(6, *CUBES_8[1], "fp8")
            make_cube(4)                                  # R9
            for idx, (kind, k) in enumerate(CUBES_8):     # L4, R10 -> fp8
                q = emit_stt(k, sq[5 + idx], "bf")
                emit_relu(kind, q[:], cube8[:, idx])

            # ---- matmul groups ----
            # The constant group is a rank-1 broadcast: one K=1 matmul per
            # PSUM bank seeds it with the precomputed column-sum bias.
            for m in range(M):
                nc.tensor.matmul(psm[m][:], ones[0:1, :], brow[:],
                                 start=True, stop=False)

            def emit_mm(lhs_of, dq_of, last):
                for m in range(M):
                    for g in range(G):
                        nc.tensor.matmul(
                            psm[m][:], lhs_of(m, g), dq_of(g),
                            start=False, stop=(last and g == G - 1),
                        )

            emit_mm(lambda m, g: xt[:, g, m * 128:(m + 1) * 128],
                    lambda g: dqP1[:, g, :], False)
            emit_mm(lambda m, g: p2[:, g, m * 128:(m + 1) * 128],
                    lambda g: dqF[:, 0, g, :], False)
            emit_mm(lambda m, g: p3[:, g, m * 128:(m + 1) * 128],
                    lambda g: dqF[:, 1, g, :], False)
            dq_of = [lambda g: dqB[:, 0, g, :], lambda g: dqF[:, 2, g, :],
                     lambda g: dqB[:, 1, g, :], lambda g: dqB[:, 2, g, :],
                     lambda g: dqB[:, 3, g, :]]
            for ci in range(len(CUBES_R) - 1):
                cube = cube_tiles[ci]
                emit_mm(lambda m, g, cube=cube: cube[:, g, m * 128:(m + 1) * 128],
                        dq_of[ci], False)

            # fp8 DoubleRow pair into the second PSUM bank set
            tmp8s = []
            for m in range(M):
                for g in range(G):
                    nc.tensor.matmul(
                        ps8[m][:],
                        cube8[:, :, g, m * 128:(m + 1) * 128],
                        dq8[:, g, :, :],
                        start=(g == 0), stop=(g == G - 1),
                        perf_mode=DR,
                    )
                tmp8 = outp.tile([128, N_OUT], f32, name="tmp8", tag="tmp8")
                nc.scalar.activation(tmp8[:], ps8[m][:], AF.Copy,
                                     scale=2.0 ** -(A_SH + B_SH))
                tmp8s.append(tmp8)

            # last f32r group (R9) closes the main accumulation; each m's
            # merge + store chases its final matmul
            cube = cube_tiles[len(CUBES_R) - 1]
            for m in range(M):
                for g in range(G):
                    nc.tensor.matmul(
                        psm[m][:], cube[:, g, m * 128:(m + 1) * 128],
                        dq_of[len(CUBES_R) - 1](g),
                        start=False, stop=(g == G - 1),
                    )
                ot = outp.tile([128, N_OUT], f32, name="ot", tag="ot")
                nc.vector.scalar_tensor_tensor(
                    ot[:], psm[m][:], 1.0, tmp8s[m][:],
                    op0=ALU.mult, op1=ALU.add,
                )
                nc.sync.dma_start(OUT[m * 128:(m + 1) * 128, :], ot[:])

    nc.compile()
    return nc


# test-harness knobs (the grader just calls kernel())
TRACE = False
LAST_RESULTS = None


def kernel(x, grid, C, w_silu, w_sp):
    from concourse import bass_utils

    if "nc" not in _CACHE:
        _CACHE["nc"] = _build()
    nc = _CACHE["nc"]

    x = np.ascontiguousarray(np.asarray(x, dtype=np.float32))
    BROW, WP1, WF, WB, W8 = _prep_weights(np.asarray(C), np.asarray(w_silu),
                                          np.asarray(w_sp))

    in_maps = []
    for c in range(NCORES):
        xT = np.ascontiguousarray(x[c * ROWS:(c + 1) * ROWS].T).astype(ml_dtypes.bfloat16)
        in_maps.append({"xT": xT, "Brow": BROW, "Wp1": WP1, "Wf": WF, "Wb": WB, "W8": W8})

    res = bass_utils.run_bass_kernel_spmd(
        nc, in_maps, core_ids=list(range(NCORES)), trace=TRACE
    )
    global LAST_RESULTS
    LAST_RESULTS = res
    return np.concatenate([res.results[c]["out"] for c in range(NCORES)], axis=0)


# revision 29
# speedup vs baseline: 1.0460x; 1.0460x over previous
"""KAN layer (uniform cubic B-spline, grid=8, k=3) Trainium2 kernel, v3.

Math
----
Reference computes, per batch row n and output o:
    out[n,o] = sum_i w_silu[i,o]*silu(x[n,i]) + sum_i w_sp[i,o] * sum_b B_b(x[n,i]) * C[b,i,o]

With the uniform knot grid t_j = -1.75 + 0.25*j, put s = 4x+7 in [3,11). The
spline space restricted to [-1,1] is spanned by 11 truncated-power functions
{1, x, x^2, x^3} u {(s-k)_+^3/6 : k=7..10} u {(k-s)_+^3/6 : k=4..6} (the
two-sided split keeps every tile O(1)-bounded so reduced-precision matmul
stays well-conditioned). silu(x) itself is least-squares fitted in the SAME
basis (max abs fit err 2.9e-6), so w_silu folds into the 11 weight groups and
needs no group of its own. The device computes 11 activation tiles and
contracts them against (n_in, n_out) weight groups on the PE, fp32 PSUM.

Every cube tile is produced sign-positive in three ops with no clamp op:
    t2 = (s-k)^2/6          [ACT Square with affine]
    q  = (s-k) * t2         [DVE scalar_tensor_tensor]  ( = (s-k)^3/6 )
    cube = relu(+-q)        [ACT Relu, scale=-1 for left knots]

Mixed precision (host-simulated end-to-end rel err 1.31e-2 vs 2e-2 budget):
  - x shipped bf16; P0 (ones) and P1 (=x) groups bf16.
  - P2, P3, R7 groups fp32r (1 PE cycle/row, same speed as bf16).
  - L6, R8, L5, R9 cube chains + weights fully bf16 (2x elementwise
    throughput, half the weight DMA).
  - Tiny cubes L4+R10 packed into ONE fp8e4m3 DoubleRow matmul group
    (two contraction planes per instruction), tiles scaled 2^10, weights
    2^12, accumulated in a second PSUM bank set, merged at the end as
    out = psum_main + 2^-22 * psum_fp8.

Schedule: all weight DMAs issued up-front to persistent SBUF tiles in
first-use order (no buffer-recycle stalls). `ones` needs no DMA and a dozen
bf16 scratch matmuls warm the PE p-state during the DMA lead-in. Engine
queues are strict FIFO, so ACT/DVE emission order is interleaved to land
each tile just before its matmul group. Output row-chunks are merged and
DMA'd as soon as their last matmul retires.

Sharding: data-parallel over batch N across 8 cores (512 rows each);
weights replicated. No collectives.
"""

import numpy as np
import ml_dtypes

N, N_IN, N_OUT = 4096, 512, 512
NB = 11
NCORES = 8
ROWS = N // NCORES          # batch rows per core
G = N_IN // 128             # 4 partition groups over n_in
M = ROWS // 128             # 4 PSUM row-chunks
W5 = (1.0, -4.0, 6.0, -4.0, 1.0)

# silu(x) ~ lstsq fit in the truncated-power basis
# [1, x, x^2, x^3, L4, L5, L6, R7, R8, R9, R10]  (max abs err 2.9e-6)
SILU_COEF = np.array([
    -2.85017504e-06, 5.00000000e-01, 2.51316134e-01, 1.04215478e-02,
    -6.77741053e-04, -1.30248882e-03, -1.77424080e-03, -1.95404022e-03,
    -1.77424080e-03, -1.30248882e-03, -6.77741053e-04,
])

# cube groups in matmul-group order: (kind, knot, chain dtype)
CUBES_R = [("L", 6, "bf"), ("R", 7, "f32"), ("R", 8, "bf"), ("L", 5, "bf"), ("R", 9, "bf")]
CUBES_8 = [("L", 4), ("R", 10)]   # fp8 DoubleRow pair
A_SH = 10   # fp8 tile scale 2^A_SH
B_SH = 12   # fp8 weight scale 2^B_SH
N_WARMUP = 7

_CACHE = {}


def _fp32r(a):
    """Round float32 array to fp32r (11-bit mantissa, RNE) — matches the
    walrus fp32_to_fp32r semantics so device-side rounding is a no-op."""
    a = np.ascontiguousarray(a, dtype=np.float32)
    bits = a.view(np.uint32)
    rnd = ((bits >> np.uint32(12)) & np.uint32(1)) + np.uint32(0x7FF)
    return ((bits + rnd) & np.uint32(0xFFFFF000)).view(np.float32)


def _poly_alpha():
    """alpha[j, t]: coefficient of x^t in the polynomial part of B_j."""
    alpha = np.zeros((NB, 4), dtype=np.float64)
    for j in range(NB):
        for p in range(5):
            k = j + p
            if k <= 6:  # (s-k)^3/6 with s-k = 4x + (7-k)
                a = 7.0 - k
                alpha[j, 3] += W5[p] * 64.0 / 6.0
                alpha[j, 2] += W5[p] * 48.0 * a / 6.0
                alpha[j, 1] += W5[p] * 12.0 * a * a / 6.0
                alpha[j, 0] += W5[p] * a * a * a / 6.0
    return alpha


def _prep_weights(C, w_silu, w_sp):
    """Fold C*w_sp and the silu fit into the 11 weight groups.
    Returns (BROW bf16 [1,n_out] = colsum of the constant group, WP1 bf16,
    WF f32r [P2,P3,R7], WB bf16 [L6,R8,L5,R9], W8 fp8 [n_in, 2, n_out])."""
    Ceff = C.astype(np.float64) * w_sp.astype(np.float64)[None]
    ws = w_silu.astype(np.float64)
    alpha = _poly_alpha()
    beta = np.einsum("jt,jio->tio", alpha, Ceff)  # (4, n_in, n_out)
    Wp = [beta[t] + SILU_COEF[t] * ws for t in range(4)]

    cube_order = [("L", 4), ("L", 5), ("L", 6), ("R", 7), ("R", 8), ("R", 9), ("R", 10)]
    Wc = {}
    for gi, (kind, k) in enumerate(cube_order):
        # device tiles are (+-(s-k))_+^3/6, always positive: no sign flips
        wk = np.zeros((N_IN, N_OUT), dtype=np.float64)
        for p in range(5):
            j = k - p
            if 0 <= j < NB:
                wk += W5[p] * Ceff[j]
        Wc[(kind, k)] = wk + SILU_COEF[4 + gi] * ws

    bf = ml_dtypes.bfloat16
    BROW = Wp[0].sum(axis=0, keepdims=True).astype(np.float32).astype(bf)
    WP1 = Wp[1].astype(np.float32).astype(bf)
    WF = _fp32r(np.stack([Wp[2], Wp[3], Wc[("R", 7)]]).astype(np.float32))
    WB = np.stack([Wc[("L", 6)], Wc[("R", 8)], Wc[("L", 5)], Wc[("R", 9)]])
    WB = WB.astype(np.float32).astype(bf)
    W8 = np.empty((N_IN, 2, N_OUT), dtype=np.float32)
    W8[:, 0] = Wc[CUBES_8[0]] * 2.0 ** B_SH
    W8[:, 1] = Wc[CUBES_8[1]] * 2.0 ** B_SH
    W8 = W8.astype(ml_dtypes.float8_e4m3)
    return BROW, WP1, WF, WB, W8


def _build():
    import concourse.bacc as bacc
    import concourse.mybir as mybir
    from concourse import tile

    f32 = mybir.dt.float32
    f32r = mybir.dt.float32r
    bf16 = mybir.dt.bfloat16
    fp8 = mybir.dt.float8e4
    AF = mybir.ActivationFunctionType
    ALU = mybir.AluOpType
    DR = mybir.MatmulPerfMode.DoubleRow

    c2 = 6.0 ** (-0.5)          # sqrt scaling: t2 = (c2*(s-k))^2
    s8 = 2.0 ** (A_SH / 2)      # extra sqrt scale for the fp8 pair

    nc = bacc.Bacc("TRN2", target_bir_lowering=False, debug=False)
    XT = nc.dram_tensor("xT", [N_IN, ROWS], bf16, kind="ExternalInput").ap()
    TB = nc.dram_tensor("Brow", [1, N_OUT], bf16, kind="ExternalInput").ap()
    TP1 = nc.dram_tensor("Wp1", [N_IN, N_OUT], bf16, kind="ExternalInput").ap()
    TWF = nc.dram_tensor("Wf", [3, N_IN, N_OUT], f32r, kind="ExternalInput").ap()
    TWB = nc.dram_tensor("Wb", [4, N_IN, N_OUT], bf16, kind="ExternalInput").ap()
    TW8 = nc.dram_tensor("W8", [N_IN, 2, N_OUT], fp8, kind="ExternalInput").ap()
    OUT = nc.dram_tensor("out", [ROWS, N_OUT], f32, kind="ExternalOutput").ap()

    with tile.TileContext(nc) as tc:
        with (
            tc.tile_pool(name="const", bufs=1) as constp,
            tc.tile_pool(name="t2f", bufs=1) as t2fp,
            tc.tile_pool(name="t2b", bufs=4) as t2bp,
            tc.tile_pool(name="qf", bufs=1) as qfp,
            tc.tile_pool(name="qb", bufs=3) as qbp,
            tc.tile_pool(name="cubef", bufs=1) as cubefp,
            tc.tile_pool(name="cubeb", bufs=3) as cubebp,
            tc.tile_pool(name="outp", bufs=6) as outp,
            tc.tile_pool(name="psp", bufs=1, space="PSUM") as psp,
        ):
            # ---- persistent tiles ----
            xt = constp.tile([128, G, ROWS], bf16)
            brow = constp.tile([1, N_OUT], bf16)
            dqP1 = constp.tile([128, G, N_OUT], bf16)
            dqF = constp.tile([128, 3, G, N_OUT], f32r)
            dqB = constp.tile([128, 4, G, N_OUT], bf16)
            dq8 = constp.tile([128, G, 2, N_OUT], fp8)
            ones = constp.tile([128, 128], bf16)
            scratch = constp.tile([128, N_OUT], f32)
            scr16 = constp.tile([128, N_OUT], bf16)
            U = constp.tile([128, G, ROWS], f32)
            p2 = constp.tile([128, G, ROWS], f32r)
            p3 = constp.tile([128, G, ROWS], f32r)
            cube8 = constp.tile([128, 2, G, ROWS], fp8)

            # ---- all DMAs up-front, in first-use order. Granularity: one
            # dma_start per weight PLANE — a dma_start's semaphore fires only
            # when the whole transfer lands, so coarser leaves matmul groups
            # waiting on later planes, while finer wastes ~600ns of Sync
            # issue time per extra instruction ----
            nc.sync.dma_start(brow[:], TB[:])
            nc.sync.dma_start(xt[:], XT.rearrange("(g p) n -> p g n", p=128))
            nc.sync.dma_start(dqP1[:], TP1.rearrange("(g p) o -> p g o", p=128))
            nc.sync.dma_start(dqF[:, 0], TWF[0].rearrange("(g p) o -> p g o", p=128))
            nc.sync.dma_start(dqF[:, 1], TWF[1].rearrange("(g p) o -> p g o", p=128))
            nc.sync.dma_start(dqB[:, 0], TWB[0].rearrange("(g p) o -> p g o", p=128))
            nc.sync.dma_start(dqF[:, 2], TWF[2].rearrange("(g p) o -> p g o", p=128))
            for j in (1, 2, 3):
                nc.sync.dma_start(dqB[:, j], TWB[j].rearrange("(g p) o -> p g o", p=128))
            nc.sync.dma_start(dq8[:], TW8.rearrange("(g p) two o -> p g two o", p=128))

            # ---- constants (no DMA deps) + PE warm-up ----
            nc.gpsimd.memset(scratch[:], 0.0)
            # bf16/f32r memset fails the walrus ISA check; derive from scratch
            nc.vector.tensor_scalar(ones[:], scratch[:, 0:128], 0.0, 1.0,
                                    op0=ALU.mult, op1=ALU.add)
            nc.vector.tensor_scalar(scr16[:], scratch[:], 0.0, 0.0,
                                    op0=ALU.mult, op1=ALU.add)
            # ACT Square bias columns: 5 f32r/bf16 cubes then the 2 fp8 cubes
            nbias = len(CUBES_R) + len(CUBES_8)
            bias_t2 = constp.tile([128, nbias], f32)
            for ci, (kind, k, _dt) in enumerate(CUBES_R):
                nc.gpsimd.memset(bias_t2[:, ci:ci + 1], (7.0 - k) * c2)
            for idx, (kind, k) in enumerate(CUBES_8):
                nc.gpsimd.memset(bias_t2[:, 5 + idx:5 + idx + 1], (7.0 - k) * c2 * s8)

            psm = [psp.tile([128, N_OUT], f32, name=f"ps{m}", tag=f"ps{m}") for m in range(M)]
            ps8 = [psp.tile([128, N_OUT], f32, name=f"q{m}", tag=f"q{m}") for m in range(M)]

            for _ in range(N_WARMUP):
                nc.tensor.matmul(ps8[M - 1][:], ones[:], scr16[:],
                                 start=True, stop=True)

            # ---- activation tiles (FIFO engines: order is the schedule) ----
            cube_specs = {}  # ci -> (kind, k, t2 tile, q pool, cube pool/dtype)

            def emit_sq(bcol, kind, k, dt):
                pool = t2fp if dt == "f32" else t2bp
                t2 = pool.tile([128, G, ROWS], f32 if dt == "f32" else bf16,
                               name="t2", tag=f"t2{dt}")
                sc = 4.0 * c2 * (s8 if dt == "fp8" else 1.0)
                nc.scalar.activation(t2[:], xt[:], AF.Square,
                                     bias=bias_t2[:, bcol:bcol + 1], scale=sc)
                return t2

            def emit_stt(k, t2, dt):
                pool = qfp if dt == "f32" else qbp
                q = pool.tile([128, G, ROWS], f32 if dt == "f32" else bf16,
                              name="q", tag=f"q{dt}")
                nc.vector.scalar_tensor_tensor(q[:], U[:], -float(k), t2[:],
                                               op0=ALU.add, op1=ALU.mult)
                return q

            def emit_relu(kind, q, dst):
                nc.scalar.activation(dst, q[:], AF.Relu,
                                     scale=(-1.0 if kind == "L" else 1.0))

            # ACT: p2 chunks; DVE: U
            for g in range(G):
                nc.scalar.activation(p2[:, g, :], xt[:, g, :], AF.Square)
            nc.vector.tensor_scalar(U[:], xt[:], 4.0, 7.0, op0=ALU.mult, op1=ALU.add)

            # ACT: first squares while DVE does p3
            sq = {}
            sq[0] = emit_sq(0, *CUBES_R[0][:2], CUBES_R[0][2])
            sq[1] = emit_sq(1, *CUBES_R[1][:2], CUBES_R[1][2])
            for g in range(G):
                nc.vector.tensor_tensor(p3[:, g, :], p2[:, g, :], xt[:, g, :], op=ALU.mult)
            sq[2] = emit_sq(2, *CUBES_R[2][:2], CUBES_R[2][2])
            sq[3] = emit_sq(3, *CUBES_R[3][:2], CUBES_R[3][2])

            # DVE stt + ACT relu pipeline, interleaved with remaining squares
            cube_tiles = [None] * len(CUBES_R)

            def make_cube(ci):
                kind, k, dt = CUBES_R[ci]
                q = emit_stt(k, sq[ci], dt)
                pool = cubefp if dt == "f32" else cubebp
                cube = pool.tile([128, G, ROWS], f32r if dt == "f32" else bf16,
                                 name="cube", tag=f"cube{dt}")
                emit_relu(kind, q[:], cube[:])
                cube_tiles[ci] = cube

            make_cube(0)                                  # L6
            make_cube(1)                                  # R7
            sq[4] = emit_sq(4, *CUBES_R[4][:2], CUBES_R[4][2])
            make_cube(2)                                  # R8
            sq[5] = emit_sq(5, *CUBES_8[0], "fp8")
            make_cube(3)                                  # L5
            sq[6] = emit_sq(6, *CUBES_8[1], "fp8")
            make_cube(4)                                  # R9
            for idx, (kind, k) in enumerate(CUBES_8):     # L4, R10 -> fp8
                q = emit_stt(k, sq[5 + idx], "bf")
                emit_relu(kind, q[:], cube8[:, idx])

            # ---- matmul groups ----
            # The constant group is a rank-1 broadcast: one K=1 matmul per
            # PSUM bank seeds it with the precomputed column-sum bias.
            for m in range(M):
                nc.tensor.matmul(psm[m][:], ones[0:1, :], brow[:],
                                 start=True, stop=False)

            def emit_mm(lhs_of, dq_of, last):
                for m in range(M):
                    for g in range(G):
                        nc.tensor.matmul(
                            psm[m][:], lhs_of(m, g), dq_of(g),
                            start=False, stop=(last and g == G - 1),
                        )

            emit_mm(lambda m, g: xt[:, g, m * 128:(m + 1) * 128],
                    lambda g: dqP1[:, g, :], False)
            emit_mm(lambda m, g: p2[:, g, m * 128:(m + 1) * 128],
                    lambda g: dqF[:, 0, g, :], False)
            emit_mm(lambda m, g: p3[:, g, m * 128:(m + 1) * 128],
                    lambda g: dqF[:, 1, g, :], False)
            dq_of = [lambda g: dqB[:, 0, g, :], lambda g: dqF[:, 2, g, :],
                     lambda g: dqB[:, 1, g, :], lambda g: dqB[:, 2, g, :],
                     lambda g: dqB[:, 3, g, :]]
            for ci in range(len(CUBES_R) - 1):
                cube = cube_tiles[ci]
                emit_mm(lambda m, g, cube=cube: cube[:, g, m * 128:(m + 1) * 128],
                        dq_of[ci], False)

            # fp8 DoubleRow pair into the second PSUM bank set
            tmp8s = []
            for m in range(M):
                for g in range(G):
                    nc.tensor.matmul(
                        ps8[m][:],
                        cube8[:, :, g, m * 128:(m + 1) * 128],
                        dq8[:, g, :, :],
                        start=(g == 0), stop=(g == G - 1),
                        perf_mode=DR,
                    )
                tmp8 = outp.tile([128, N_OUT], f32, name="tmp8", tag="tmp8")
                nc.scalar.activation(tmp8[:], ps8[m][:], AF.Copy,
                                     scale=2.0 ** -(A_SH + B_SH))
                tmp8s.append(tmp8)

            # last f32r group (R9) closes the main accumulation; each m's
            # merge + store chases its final matmul
            cube = cube_tiles[len(CUBES_R) - 1]
            for m in range(M):
                for g in range(G):
                    nc.tensor.matmul(
                        psm[m][:], cube[:, g, m * 128:(m + 1) * 128],
                        dq_of[len(CUBES_R) - 1](g),
                        start=False, stop=(g == G - 1),
                    )
                ot = outp.tile([128, N_OUT], f32, name="ot", tag="ot")
                nc.vector.scalar_tensor_tensor(
                    ot[:], psm[m][:], 1.0, tmp8s[m][:],
                    op0=ALU.mult, op1=ALU.add,
                )
                nc.sync.dma_start(OUT[m * 128:(m + 1) * 128, :], ot[:])

    nc.compile()
    return nc


# test-harness knobs (the grader just calls kernel())
TRACE = False
LAST_RESULTS = None


def kernel(x, grid, C, w_silu, w_sp):
    from concourse import bass_utils

    if "nc" not in _CACHE:
        _CACHE["nc"] = _build()
    nc = _CACHE["nc"]

    x = np.ascontiguousarray(np.asarray(x, dtype=np.float32))
    BROW, WP1, WF, WB, W8 = _prep_weights(np.asarray(C), np.asarray(w_silu),
                                          np.asarray(w_sp))

    in_maps = []
    for c in range(NCORES):
        xT = np.ascontiguousarray(x[c * ROWS:(c + 1) * ROWS].T).astype(ml_dtypes.bfloat16)
        in_maps.append({"xT": xT, "Brow": BROW, "Wp1": WP1, "Wf": WF, "Wb": WB, "W8": W8})

    res = bass_utils.run_bass_kernel_spmd(
        nc, in_maps, core_ids=list(range(NCORES)), trace=TRACE
    )
    global LAST_RESULTS
    LAST_RESULTS = res
    return np.concatenate([res.results[c]["out"] for c in range(NCORES)], axis=0)


# revision 30
# speedup vs baseline: 1.0623x; 1.0155x over previous
"""KAN layer (uniform cubic B-spline, grid=8, k=3) Trainium2 kernel, v3.

Math
----
Reference computes, per batch row n and output o:
    out[n,o] = sum_i w_silu[i,o]*silu(x[n,i]) + sum_i w_sp[i,o] * sum_b B_b(x[n,i]) * C[b,i,o]

With the uniform knot grid t_j = -1.75 + 0.25*j, put s = 4x+7 in [3,11). The
spline space restricted to [-1,1] is spanned by 11 truncated-power functions
{1, x, x^2, x^3} u {(s-k)_+^3/6 : k=7..10} u {(k-s)_+^3/6 : k=4..6} (the
two-sided split keeps every tile O(1)-bounded so reduced-precision matmul
stays well-conditioned). silu(x) itself is least-squares fitted in the SAME
basis (max abs fit err 2.9e-6), so w_silu folds into the 11 weight groups and
needs no group of its own. The device computes 11 activation tiles and
contracts them against (n_in, n_out) weight groups on the PE, fp32 PSUM.

Every cube tile is produced sign-positive in three ops with no clamp op:
    t2 = (s-k)^2/6          [ACT Square with affine]
    q  = (s-k) * t2         [DVE scalar_tensor_tensor]  ( = (s-k)^3/6 )
    cube = relu(+-q)        [ACT Relu, scale=-1 for left knots]

Mixed precision (host-simulated end-to-end rel err 1.31e-2 vs 2e-2 budget):
  - x shipped bf16; P0 (ones) and P1 (=x) groups bf16.
  - P2, P3, R7 groups fp32r (1 PE cycle/row, same speed as bf16).
  - L6, R8, L5, R9 cube chains + weights fully bf16 (2x elementwise
    throughput, half the weight DMA).
  - Tiny cubes L4+R10 packed into ONE fp8e4m3 DoubleRow matmul group
    (two contraction planes per instruction), tiles scaled 2^10, weights
    2^12, accumulated in a second PSUM bank set, merged at the end as
    out = psum_main + 2^-22 * psum_fp8.

Schedule: all weight DMAs issued up-front to persistent SBUF tiles in
first-use order (no buffer-recycle stalls). `ones` needs no DMA and a dozen
bf16 scratch matmuls warm the PE p-state during the DMA lead-in. Engine
queues are strict FIFO, so ACT/DVE emission order is interleaved to land
each tile just before its matmul group. Output row-chunks are merged and
DMA'd as soon as their last matmul retires.

Sharding: data-parallel over batch N across 8 cores (512 rows each);
weights replicated. No collectives.
"""

import numpy as np
import ml_dtypes

N, N_IN, N_OUT = 4096, 512, 512
NB = 11
NCORES = 8
ROWS = N // NCORES          # batch rows per core
G = N_IN // 128             # 4 partition groups over n_in
M = ROWS // 128             # 4 PSUM row-chunks
W5 = (1.0, -4.0, 6.0, -4.0, 1.0)

# silu(x) ~ lstsq fit in the truncated-power basis
# [1, x, x^2, x^3, L4, L5, L6, R7, R8, R9, R10]  (max abs err 2.9e-6)
SILU_COEF = np.array([
    -2.85017504e-06, 5.00000000e-01, 2.51316134e-01, 1.04215478e-02,
    -6.77741053e-04, -1.30248882e-03, -1.77424080e-03, -1.95404022e-03,
    -1.77424080e-03, -1.30248882e-03, -6.77741053e-04,
])

# cube groups in matmul-group order: (kind, knot, chain dtype)
CUBES_R = [("L", 6, "bf"), ("R", 7, "f32"), ("R", 8, "bf"), ("L", 5, "bf"), ("R", 9, "bf")]
CUBES_8 = [("L", 4), ("R", 10)]   # fp8 DoubleRow pair
A_SH = 10   # fp8 tile scale 2^A_SH
B_SH = 12   # fp8 weight scale 2^B_SH
N_WARMUP = 7

_CACHE = {}


def _fp32r(a):
    """Round float32 array to fp32r (11-bit mantissa, RNE) — matches the
    walrus fp32_to_fp32r semantics so device-side rounding is a no-op."""
    a = np.ascontiguousarray(a, dtype=np.float32)
    bits = a.view(np.uint32)
    rnd = ((bits >> np.uint32(12)) & np.uint32(1)) + np.uint32(0x7FF)
    return ((bits + rnd) & np.uint32(0xFFFFF000)).view(np.float32)


def _poly_alpha():
    """alpha[j, t]: coefficient of x^t in the polynomial part of B_j."""
    alpha = np.zeros((NB, 4), dtype=np.float64)
    for j in range(NB):
        for p in range(5):
            k = j + p
            if k <= 6:  # (s-k)^3/6 with s-k = 4x + (7-k)
                a = 7.0 - k
                alpha[j, 3] += W5[p] * 64.0 / 6.0
                alpha[j, 2] += W5[p] * 48.0 * a / 6.0
                alpha[j, 1] += W5[p] * 12.0 * a * a / 6.0
                alpha[j, 0] += W5[p] * a * a * a / 6.0
    return alpha


def _prep_weights(C, w_silu, w_sp):
    """Fold C*w_sp and the silu fit into the 11 weight groups.
    Returns (BROW bf16 [1,n_out] = colsum of the constant group, WP1 bf16,
    WF f32r [P2,P3,R7], WB bf16 [L6,R8,L5,R9], W8 fp8 [n_in, 2, n_out])."""
    Ceff = C.astype(np.float64) * w_sp.astype(np.float64)[None]
    ws = w_silu.astype(np.float64)
    alpha = _poly_alpha()
    beta = np.einsum("jt,jio->tio", alpha, Ceff)  # (4, n_in, n_out)
    Wp = [beta[t] + SILU_COEF[t] * ws for t in range(4)]

    cube_order = [("L", 4), ("L", 5), ("L", 6), ("R", 7), ("R", 8), ("R", 9), ("R", 10)]
    Wc = {}
    for gi, (kind, k) in enumerate(cube_order):
        # device tiles are (+-(s-k))_+^3/6, always positive: no sign flips
        wk = np.zeros((N_IN, N_OUT), dtype=np.float64)
        for p in range(5):
            j = k - p
            if 0 <= j < NB:
                wk += W5[p] * Ceff[j]
        Wc[(kind, k)] = wk + SILU_COEF[4 + gi] * ws

    bf = ml_dtypes.bfloat16
    BROW = Wp[0].sum(axis=0, keepdims=True).astype(np.float32).astype(bf)
    WP1 = Wp[1].astype(np.float32).astype(bf)
    WF = _fp32r(np.stack([Wp[2], Wp[3], Wc[("R", 7)]]).astype(np.float32))
    WB = np.stack([Wc[("L", 6)], Wc[("R", 8)], Wc[("L", 5)], Wc[("R", 9)]])
    WB = WB.astype(np.float32).astype(bf)
    W8 = np.empty((N_IN, 2, N_OUT), dtype=np.float32)
    W8[:, 0] = Wc[CUBES_8[0]] * 2.0 ** B_SH
    W8[:, 1] = Wc[CUBES_8[1]] * 2.0 ** B_SH
    W8 = W8.astype(ml_dtypes.float8_e4m3)
    return BROW, WP1, WF, WB, W8


def _build():
    import concourse.bacc as bacc
    import concourse.mybir as mybir
    from concourse import tile

    f32 = mybir.dt.float32
    f32r = mybir.dt.float32r
    bf16 = mybir.dt.bfloat16
    fp8 = mybir.dt.float8e4
    AF = mybir.ActivationFunctionType
    ALU = mybir.AluOpType
    DR = mybir.MatmulPerfMode.DoubleRow

    c2 = 6.0 ** (-0.5)          # sqrt scaling: t2 = (c2*(s-k))^2
    s8 = 2.0 ** (A_SH / 2)      # extra sqrt scale for the fp8 pair

    nc = bacc.Bacc("TRN2", target_bir_lowering=False, debug=False)
    XT = nc.dram_tensor("xT", [N_IN, ROWS], bf16, kind="ExternalInput").ap()
    TB = nc.dram_tensor("Brow", [1, N_OUT], bf16, kind="ExternalInput").ap()
    TP1 = nc.dram_tensor("Wp1", [N_IN, N_OUT], bf16, kind="ExternalInput").ap()
    TWF = nc.dram_tensor("Wf", [3, N_IN, N_OUT], f32r, kind="ExternalInput").ap()
    TWB = nc.dram_tensor("Wb", [4, N_IN, N_OUT], bf16, kind="ExternalInput").ap()
    TW8 = nc.dram_tensor("W8", [N_IN, 2, N_OUT], fp8, kind="ExternalInput").ap()
    OUT = nc.dram_tensor("out", [ROWS, N_OUT], f32, kind="ExternalOutput").ap()

    with tile.TileContext(nc) as tc:
        with (
            tc.tile_pool(name="const", bufs=1) as constp,
            tc.tile_pool(name="t2f", bufs=1) as t2fp,
            tc.tile_pool(name="t2b", bufs=4) as t2bp,
            tc.tile_pool(name="qf", bufs=1) as qfp,
            tc.tile_pool(name="qb", bufs=3) as qbp,
            tc.tile_pool(name="cubef", bufs=1) as cubefp,
            tc.tile_pool(name="cubeb", bufs=3) as cubebp,
            tc.tile_pool(name="outp", bufs=6) as outp,
            tc.tile_pool(name="psp", bufs=1, space="PSUM") as psp,
        ):
            # ---- persistent tiles ----
            xt = constp.tile([128, G, ROWS], bf16)
            brow = constp.tile([1, N_OUT], bf16)
            dqP1 = constp.tile([128, G, N_OUT], bf16)
            dqF = constp.tile([128, 3, G, N_OUT], f32r)
            dqB = constp.tile([128, 4, G, N_OUT], bf16)
            dq8 = constp.tile([128, G, 2, N_OUT], fp8)
            ones = constp.tile([128, 128], bf16)
            scratch = constp.tile([128, N_OUT], f32)
            scr16 = constp.tile([128, N_OUT], bf16)
            U = constp.tile([128, G, ROWS], f32)
            p2 = constp.tile([128, G, ROWS], f32r)
            p3 = constp.tile([128, G, ROWS], f32r)
            cube8 = constp.tile([128, 2, G, ROWS], fp8)

            # ---- all DMAs up-front, in first-use order. Granularity: one
            # dma_start per weight PLANE — a dma_start's semaphore fires only
            # when the whole transfer lands, so coarser leaves matmul groups
            # waiting on later planes, while finer wastes ~600ns of Sync
            # issue time per extra instruction ----
            nc.sync.dma_start(brow[:], TB[:])
            nc.sync.dma_start(xt[:], XT.rearrange("(g p) n -> p g n", p=128))
            nc.sync.dma_start(dqP1[:], TP1.rearrange("(g p) o -> p g o", p=128))
            nc.sync.dma_start(dqF[:, 0], TWF[0].rearrange("(g p) o -> p g o", p=128))
            nc.sync.dma_start(dqF[:, 1], TWF[1].rearrange("(g p) o -> p g o", p=128))
            nc.sync.dma_start(dqB[:, 0], TWB[0].rearrange("(g p) o -> p g o", p=128))
            nc.sync.dma_start(dqF[:, 2], TWF[2].rearrange("(g p) o -> p g o", p=128))
            for j in (1, 2, 3):
                nc.sync.dma_start(dqB[:, j], TWB[j].rearrange("(g p) o -> p g o", p=128))
            nc.sync.dma_start(dq8[:], TW8.rearrange("(g p) two o -> p g two o", p=128))

            # ---- constants (no DMA deps) + PE warm-up ----
            nc.gpsimd.memset(scratch[:], 0.0)
            # bf16/f32r memset fails the walrus ISA check; derive from scratch
            nc.vector.tensor_scalar(ones[:], scratch[:, 0:128], 0.0, 1.0,
                                    op0=ALU.mult, op1=ALU.add)
            nc.vector.tensor_scalar(scr16[:], scratch[:], 0.0, 0.0,
                                    op0=ALU.mult, op1=ALU.add)
            # ACT Square bias columns: 5 f32r/bf16 cubes then the 2 fp8 cubes
            nbias = len(CUBES_R) + len(CUBES_8)
            bias_t2 = constp.tile([128, nbias], f32)
            for ci, (kind, k, _dt) in enumerate(CUBES_R):
                nc.gpsimd.memset(bias_t2[:, ci:ci + 1], (7.0 - k) * c2)
            for idx, (kind, k) in enumerate(CUBES_8):
                nc.gpsimd.memset(bias_t2[:, 5 + idx:5 + idx + 1], (7.0 - k) * c2 * s8)

            psm = [psp.tile([128, N_OUT], f32, name=f"ps{m}", tag=f"ps{m}") for m in range(M)]
            ps8 = [psp.tile([128, N_OUT], f32, name=f"q{m}", tag=f"q{m}") for m in range(M)]

            for _ in range(N_WARMUP):
                nc.tensor.matmul(ps8[M - 1][:], ones[:], scr16[:],
                                 start=True, stop=True)

            # ---- activation tiles (FIFO engines: order is the schedule) ----
            cube_specs = {}  # ci -> (kind, k, t2 tile, q pool, cube pool/dtype)

            def emit_sq(bcol, kind, k, dt):
                pool = t2fp if dt == "f32" else t2bp
                t2 = pool.tile([128, G, ROWS], f32 if dt == "f32" else bf16,
                               name="t2", tag=f"t2{dt}")
                sc = 4.0 * c2 * (s8 if dt == "fp8" else 1.0)
                nc.scalar.activation(t2[:], xt[:], AF.Square,
                                     bias=bias_t2[:, bcol:bcol + 1], scale=sc)
                return t2

            def emit_stt(k, t2, dt):
                pool = qfp if dt == "f32" else qbp
                q = pool.tile([128, G, ROWS], f32 if dt == "f32" else bf16,
                              name="q", tag=f"q{dt}")
                nc.vector.scalar_tensor_tensor(q[:], U[:], -float(k), t2[:],
                                               op0=ALU.add, op1=ALU.mult)
                return q

            def emit_relu(kind, q, dst):
                nc.scalar.activation(dst, q[:], AF.Relu,
                                     scale=(-1.0 if kind == "L" else 1.0))

            # ACT: p2 chunks; DVE: U
            for g in range(G):
                nc.scalar.activation(p2[:, g, :], xt[:, g, :], AF.Square)
            nc.vector.tensor_scalar(U[:], xt[:], 4.0, 7.0, op0=ALU.mult, op1=ALU.add)

            # ACT: first squares while DVE does p3
            sq = {}
            sq[0] = emit_sq(0, *CUBES_R[0][:2], CUBES_R[0][2])
            sq[1] = emit_sq(1, *CUBES_R[1][:2], CUBES_R[1][2])
            for g in range(G):
                nc.vector.tensor_tensor(p3[:, g, :], p2[:, g, :], xt[:, g, :], op=ALU.mult)
            sq[2] = emit_sq(2, *CUBES_R[2][:2], CUBES_R[2][2])
            sq[3] = emit_sq(3, *CUBES_R[3][:2], CUBES_R[3][2])

            # DVE stt + ACT relu pipeline, interleaved with remaining squares
            cube_tiles = [None] * len(CUBES_R)

            def make_cube(ci):
                kind, k, dt = CUBES_R[ci]
                q = emit_stt(k, sq[ci], dt)
                pool = cubefp if dt == "f32" else cubebp
                cube = pool.tile([128, G, ROWS], f32r if dt == "f32" else bf16,
                                 name="cube", tag=f"cube{dt}")
                emit_relu(kind, q[:], cube[:])
                cube_tiles[ci] = cube

            make_cube(0)                                  # L6
            make_cube(1)                                  # R7
            sq[4] = emit_sq(4, *CUBES_R[4][:2], CUBES_R[4][2])
            make_cube(2)                                  # R8
            sq[5] = emit_sq(5, *CUBES_8[0], "fp8")
            make_cube(3)                                  # L5
            sq[6] = emit_sq(6, *CUBES_8[1], "fp8")
            make_cube(4)                                  # R9
            for idx, (kind, k) in enumerate(CUBES_8):     # L4, R10 -> fp8
                q = emit_stt(k, sq[5 + idx], "bf")
                emit_relu(kind, q[:], cube8[:, idx])

            # ---- matmul groups ----
            # The constant group is a rank-1 broadcast: one K=1 matmul per
            # PSUM bank seeds it with the precomputed column-sum bias.
            for m in range(M):
                nc.tensor.matmul(psm[m][:], ones[0:1, :], brow[:],
                                 start=True, stop=False)

            def emit_mm(lhs_of, dq_of, last):
                for m in range(M):
                    for g in range(G):
                        nc.tensor.matmul(
                            psm[m][:], lhs_of(m, g), dq_of(g),
                            start=False, stop=(last and g == G - 1),
                        )

            emit_mm(lambda m, g: xt[:, g, m * 128:(m + 1) * 128],
                    lambda g: dqP1[:, g, :], False)
            emit_mm(lambda m, g: p2[:, g, m * 128:(m + 1) * 128],
                    lambda g: dqF[:, 0, g, :], False)
            emit_mm(lambda m, g: p3[:, g, m * 128:(m + 1) * 128],
                    lambda g: dqF[:, 1, g, :], False)
            dq_of = [lambda g: dqB[:, 0, g, :], lambda g: dqF[:, 2, g, :],
                     lambda g: dqB[:, 1, g, :], lambda g: dqB[:, 2, g, :],
                     lambda g: dqB[:, 3, g, :]]
            for ci in range(len(CUBES_R) - 1):
                cube = cube_tiles[ci]
                emit_mm(lambda m, g, cube=cube: cube[:, g, m * 128:(m + 1) * 128],
                        dq_of[ci], False)

            # fp8 DoubleRow pair into the second PSUM bank set
            tmp8s = []
            for m in range(M):
                for g in range(G):
                    nc.tensor.matmul(
                        ps8[m][:],
                        cube8[:, :, g, m * 128:(m + 1) * 128],
                        dq8[:, g, :, :],
                        start=(g == 0), stop=(g == G - 1),
                        perf_mode=DR,
                    )
                tmp8 = outp.tile([128, N_OUT], f32, name="tmp8", tag="tmp8")
                nc.scalar.activation(tmp8[:], ps8[m][:], AF.Copy,
                                     scale=2.0 ** -(A_SH + B_SH))
                tmp8s.append(tmp8)

            # last f32r group (R9) closes the main accumulation; each m's
            # merge + store chases its final matmul
            cube = cube_tiles[len(CUBES_R) - 1]
            for m in range(M):
                for g in range(G):
                    nc.tensor.matmul(
                        psm[m][:], cube[:, g, m * 128:(m + 1) * 128],
                        dq_of[len(CUBES_R) - 1](g),
                        start=False, stop=(g == G - 1),
                    )
                ot = outp.tile([128, N_OUT], f32, name="ot", tag="ot")
                for h in (slice(0, N_OUT // 2), slice(N_OUT // 2, N_OUT)):
                    nc.vector.scalar_tensor_tensor(
                        ot[:, h], psm[m][:, h], 1.0, tmp8s[m][:, h],
                        op0=ALU.mult, op1=ALU.add,
                    )
                    nc.sync.dma_start(OUT[m * 128:(m + 1) * 128, h], ot[:, h])

    nc.compile()
    return nc


# test-harness knobs (the grader just calls kernel())
TRACE = False
LAST_RESULTS = None


def kernel(x, grid, C, w_silu, w_sp):
    from concourse import bass_utils

    if "nc" not in _CACHE:
        _CACHE["nc"] = _build()
    nc = _CACHE["nc"]

    x = np.ascontiguousarray(np.asarray(x, dtype=np.float32))
    BROW, WP1, WF, WB, W8 = _prep_weights(np.asarray(C), np.asarray(w_silu),
                                          np.asarray(w_sp))

    in_maps = []
    for c in range(NCORES):
        xT = np.ascontiguousarray(x[c * ROWS:(c + 1) * ROWS].T).astype(ml_dtypes.bfloat16)
        in_maps.append({"xT": xT, "Brow": BROW, "Wp1": WP1, "Wf": WF, "Wb": WB, "W8": W8})

    res = bass_utils.run_bass_kernel_spmd(
        nc, in_maps, core_ids=list(range(NCORES)), trace=TRACE
    )
    global LAST_RESULTS
    LAST_RESULTS = res
    return np.concatenate([res.results[c]["out"] for c in range(NCORES)], axis=0)
